# revision 59
# baseline (speedup 1.0000x reference)
"""Trainium2 Bass kernel for nn_BSLoss (text-snake style OHEM loss), 8-core
data-parallel.

Strategy
--------
Host shards the batch dim (16 -> 2 per core). Global cross-level layouts:
  - gtm_all [128, 3*FT] fp8-e3m4 = [tr_all | tcl_all | train_all] (FT=525
    is the per-partition pixel count over all 3 levels; masks are exact 0/1
    in fp8), so masks are ONE fused op each over all levels.
  - cls_all [128, 4*FT] fp8-e3m4 = [tr_lo | tcl_lo | tr_hi | tcl_hi], so the
    CE logit diff / dce / exp are single global ops.
  - regression channels (map+pred) ship fp8-e3m4 channel-innermost per pixel,
    [gtx | reg] concatenated per (half-)chunk = one fat-row DMA each, split
    into ~0.4MB pieces so compute starts as soon as the first piece lands.

Device per core:
  - DVE: one fused custom op per chunk-piece computing q = 2*smooth_l1 AND a
    continuous running sum (scan) in one 1x pass; pieces chain their cumsum
    via the scan's s0 init read from the previous piece's last column. The
    per-level weighted regression totals come Abel-style from three strided
    reductions: A = sum_f w_f C[32f+15], B = sum_{f>=1} w_f C[32f-1],
    D = sum_f w_f C[32f+31]; host computes x = A-B, y = D-A.
  - ScalarE: sgn, exp, ln(1+x) (2-class CE; also ce+1 via ln(e*x+e)), the
    per-level tcl-CE totals, and densifying the cumsum page-end columns.
Device ships vout = [(ce_tr+1)*train | (ce_tr+1)*pos | ce_tcl+1] (bf16; the
tcl third written directly by ScalarE, unmasked). Host recovers the exact
pos set (vp>0), neg set (w1>0 & vp==0), counts, masked CE sums, does the
exact global top-k OHEM over the negative CE values, and the final
divisions (mirroring reference semantics, incl. the n_pos==0 fallbacks).
"""

import numpy as np
import ml_dtypes

import concourse.bacc as bacc
import concourse.mybir as mybir
import concourse.dve_ops as dve_ops
from concourse.dve_spec import (
    Spec, Src0, Src1, C0, Zero, One, AluOp, Bin, minn, scan, lower, _has_src1,
)
from concourse.dve_uop import DveOpSpec
from concourse import tile

F32 = mybir.dt.float32
BF16 = mybir.dt.bfloat16
FP8 = mybir.dt.float8e3
NP_BF16 = ml_dtypes.bfloat16
NP_FP8 = ml_dtypes.float8_e3m4
ALU = mybir.AluOpType
ACT = mybir.ActivationFunctionType

NCORES = 8
B_PER_CORE = 2
# level -> (H, W, npieces); pieces are the DMA/compute granularity
LEVELS = [(3, 160, 160, 8), (4, 80, 80, 2), (5, 40, 40, 1)]
# per-level piece pixel-counts (sum == FS[li]); first L3 pieces are small so
# compute starts as early as possible, later ones big to amortize overhead
PIECE_PIX = [[25, 25, 50, 50, 100, 100, 50], [100], [25]]
FS = [B_PER_CORE * H * W // 128 for _, H, W, _ in LEVELS]   # [400, 100, 25]
FT = sum(FS)                                                # 525
LOFF = [sum(FS[:i]) for i in range(len(LEVELS))]            # [0, 400, 500]
KCH = 16
OHEM_RATIO = 3.0

# stats: per-level tcl-CE totals (3 cols), then 3 trio cols per level
N_LEVEL_COLS = len(LEVELS)
STATS_COLS = N_LEVEL_COLS + 3 * len(LEVELS)


def _np_sl1q(d):
    a = np.abs(d)
    m = np.minimum(a, 1.0)
    return m * (a + a - m)   # == 2 * smooth_l1(d)


def _register_custom_ops():
    """Register our fused DVE ops (idempotent)."""
    # QSL1CS: out = s0 + cumsum_freedim(q(Src0 - Src1))
    a = Bin(AluOp.ABSOLUTE_DIFF, Src0, Src1)
    m = minn(a, One)
    q = ((a + a) - m) * m

    def _qs_ref(in0, in1, s0, s1, imm2):
        p = in0.shape[0]
        qq = _np_sl1q(in0.reshape(p, -1).astype(np.float32)
                      - in1.reshape(p, -1).astype(np.float32))
        init = np.asarray(s0).reshape(-1, 1) if isinstance(s0, np.ndarray) else s0
        return init + np.cumsum(qq, axis=1)

    spec_qs = Spec(body=scan(AluOp.ADD, q, init=C0), reference=_qs_ref)

    def _acc_ref(fn):
        def ref(in0, in1, s0, s1, imm2):
            p = in0.shape[0]
            o = fn(in0.reshape(p, -1).astype(np.float32),
                   in1.reshape(p, -1).astype(np.float32) if in1 is not None
                   else None)
            init = np.asarray(s0).reshape(-1, 1) if isinstance(s0, np.ndarray) else s0
            return o, init + o.sum(axis=1, keepdims=True)
        return ref

    spec_mulr = Spec(body=Src0 * Src1, accum=AluOp.ADD, accum_init=C0,
                     reference=_acc_ref(lambda a_, b_: a_ * b_))

    ops = {}
    for name, spec in (("QSL1CS_ANT", spec_qs), ("MULR_ANT", spec_mulr)):
        if name in dve_ops._SUB_OPCODE_FOR_NAME:
            ops[name] = next(o for o in dve_ops.OPS if o.name == name)
            continue
        row = dve_ops._CUSTOM_DVE_ROW_BASE + len(dve_ops.OPS)
        shas = {}
        for ver in ("v3", "v4"):
            u = lower(spec, ver=ver)
            shas[ver] = DveOpSpec(name=name, opcode=row, uops=u,
                                  rd1_en=_has_src1(spec)).sha(ver)
        op = dve_ops.DveOp(name, spec, subdim=False, uops_sha=shas)
        dve_ops.OPS.append(op)
        dve_ops.CUSTOM_DVE_SPECS[name] = spec
        dve_ops._SUB_OPCODE_FOR_NAME[name] = row
        ops[name] = op
    return ops


def _install_act_root():
    """Restrict the ACT table universe to the one set holding every function
    we use (exp, ln, identity, copy), so walrus never ping-pongs table sets."""
    import os, json, shutil, tempfile
    if os.environ.get("BASS_ACT_ROOT_JSON_PATH"):
        return
    try:
        from neuronxcc.driver.Job import Job
        from neuronxcc.driver.jobs.support.FindActInfo import findActInfoFile
        src = findActInfoFile(Job.getPackageDir(), "gen3")
        d = json.load(open(src))
        keep = [t for t in d["act_func_sets"]
                if t["name"] == "natural_log_exp_and_others"]
        if not keep:
            return
        tmp = tempfile.mkdtemp(prefix="act_root_")
        srcdir = os.path.dirname(src)
        for t in keep:
            for k in d["pwp_file_keys"]:
                shutil.copy(os.path.join(srcdir, t[k]), tmp)
        with open(os.path.join(tmp, "act_info.json"), "w") as f:
            json.dump({"pwp_file_keys": d["pwp_file_keys"],
                       "act_func_sets": keep}, f)
        os.environ["BASS_ACT_ROOT_JSON_PATH"] = os.path.join(tmp, "act_info.json")
        import concourse.hw_specs as hw_specs
        _orig_gat = hw_specs.get_activation_tables

        def _gat(module_arch):
            full = _orig_gat(module_arch)
            return {"natural_log_exp_and_others":
                    full["natural_log_exp_and_others"]}

        hw_specs.get_activation_tables = _gat
        import concourse.bacc as _bacc_mod
        _bacc_mod.get_activation_tables = _gat
        import concourse.bass_interp as _bi_mod
        _bi_mod.get_activation_tables = _gat
    except Exception:
        pass


# (level_idx, level, piece_idx, piece_pixels, pixel_offset_in_level)
def _pieces():
    out = []
    for li, (lvl, H, W, npc) in enumerate(LEVELS):
        off = 0
        for j, Fp in enumerate(PIECE_PIX[li]):
            out.append((li, lvl, j, Fp, off))
            off += Fp
        assert off == FS[li]
    return out


def build_bass():
    """Build the SPMD Bass module (one core's program)."""
    _install_act_root()
    ops = _register_custom_ops()
    nc = bacc.Bacc("TRN2")

    dram_in = {}
    dram_out = {}
    dram_in["gtma"] = nc.dram_tensor("gtma", [128, 3 * FT], FP8,
                                     kind="ExternalInput")
    dram_in["clsa"] = nc.dram_tensor("clsa", [128, 4 * FT], FP8,
                                     kind="ExternalInput")
    for li, lvl, j, Fp, off in _pieces():
        dram_in[f"ch{lvl}_{j}"] = nc.dram_tensor(
            f"ch{lvl}_{j}", [128, 64 * Fp], FP8, kind="ExternalInput")
    dram_out["vout"] = nc.dram_tensor("vout", [128, 3 * FT], BF16,
                                      kind="ExternalOutput")
    dram_out["stats"] = nc.dram_tensor(
        "stats", [128, STATS_COLS], F32, kind="ExternalOutput")

    QSL1CS, MULR = ops["QSL1CS_ANT"], ops["MULR_ANT"]

    with tile.TileContext(nc) as tc:
        with (
            tc.tile_pool(name="io", bufs=1) as io,
            tc.tile_pool(name="lv", bufs=1) as lv,
            tc.tile_pool(name="wk", bufs=1) as wk,
            tc.tile_pool(name="st", bufs=1) as stp,
        ):
            stats = stp.tile([128, STATS_COLS], F32, name="stats_t")

            GTMA = lv.tile([128, 3 * FT], FP8, tag="gtma", name="gtma_t")
            CLSA = lv.tile([128, 4 * FT], FP8, tag="clsa", name="clsa_t")
            CH = {}
            PP = {}
            for li, lvl, j, Fp, off in _pieces():
                CH[(li, j)] = io.tile([128, 64 * Fp], FP8,
                                      tag=f"ch{lvl}_{j}", name=f"ch_{lvl}_{j}")
                PP[(li, j)] = (Fp, off)

            # ---- loads: small L3 pieces first, gtm/cls interleaved
            pcs = [(0, 0), "gtm", (0, 1), (0, 2), "cls", (0, 3),
                   (0, 4), (0, 5), (0, 6), (1, 0), (2, 0)]
            for p in pcs:
                if p == "gtm":
                    nc.sync.dma_start(GTMA[:, :], dram_in["gtma"][:, :])
                elif p == "cls":
                    nc.sync.dma_start(CLSA[:, :], dram_in["clsa"][:, :])
                else:
                    li, j = p
                    lvl = LEVELS[li][0]
                    nc.sync.dma_start(CH[(li, j)][:, :],
                                      dram_in[f"ch{lvl}_{j}"][:, :])

            gtmb = lv.tile([128, 3 * FT], BF16, tag="gtmb", name="gtmb_t")
            clsb = lv.tile([128, 4 * FT], BF16, tag="clsb", name="clsb_t")
            tr_a = gtmb[:, 0:FT]
            tcl_a = gtmb[:, FT:2 * FT]
            train_a = gtmb[:, 2 * FT:3 * FT]
            pos = lv.tile([128, FT], BF16, tag="pos", name="pos_t")
            w2 = lv.tile([128, FT], F32, tag="w2", name="w2_t")
            sgn = lv.tile([128, 2 * FT], BF16, tag="sgn", name="sgn_t")
            diff = lv.tile([128, 2 * FT], BF16, tag="diff", name="diff_t")
            dce = lv.tile([128, 2 * FT], BF16, tag="dce", name="dce_t")
            expd = lv.tile([128, 2 * FT], F32, tag="expd", name="expd_t")
            cep1 = lv.tile([128, FT], BF16, tag="cep1", name="cep1_t")
            econst = lv.tile([128, 1], F32, tag="econst", name="econst_t")
            nc.gpsimd.memset(econst[:, :], float(np.e))
            # dense page-end cumsum staging (written by ScalarE copies)
            c15 = lv.tile([128, FT], F32, tag="c15", name="c15_t")
            c31 = lv.tile([128, FT], F32, tag="c31", name="c31_t")
            vout = lv.tile([128, 3 * FT], BF16, tag="vout", name="vout_t")
            QT = [wk.tile([128, 32 * FS[li]], F32, tag=f"q{li}",
                          name=f"q_{LEVELS[li][0]}") for li in range(3)]
            SCR = [wk.tile([128, FS[li]], F32, tag=f"scr{li}",
                           name=f"scr_{LEVELS[li][0]}") for li in range(3)]

            # ---- DVE: fused q+cumsum per piece (chained within level) ----
            def qpiece(li, j):
                Q = QT[li]
                Fp, off = PP[(li, j)]
                o0 = 32 * off
                s0 = 0.0 if j == 0 else Q[:, o0 - 1:o0]
                nc.vector._custom_dve(
                    QSL1CS, out=Q[:, o0:o0 + 32 * Fp],
                    in0=CH[(li, j)][:, 0:32 * Fp],
                    in1=CH[(li, j)][:, 32 * Fp:64 * Fp], s0=s0)

            # masks: one fused op each over all levels (bf16, 2x)
            def masks():
                nc.vector.tensor_mul(pos[:, :], tr_a, train_a)
                nc.vector.scalar_tensor_tensor(
                    out=w2[:, :], in0=tcl_a, scalar=1.0, in1=pos[:, :],
                    op0=ALU.add, op1=ALU.mult)

            def dd():
                nc.vector.tensor_tensor(
                    out=diff[:, :], in0=clsb[:, 2 * FT:4 * FT],
                    in1=clsb[:, 0:2 * FT], op=ALU.subtract)
                nc.vector.tensor_mul(dce[:, :], diff[:, :], sgn[:, :])

            def v_ops():
                # vout = [w1 | vp | cem1]: (ce_tr+1)*train, (ce_tr+1)*pos,
                # and raw ce_tcl+1 (written by ScalarE). Host recovers
                # pos/neg sets from vp>0 / (w1>0 & vp==0) and masked sums.
                nc.vector.tensor_mul(vout[:, 0:FT], cep1[:, :], train_a)
                nc.vector.tensor_mul(vout[:, FT:2 * FT], cep1[:, :],
                                     pos[:, :])
                nc.scalar.dma_start(dram_out["vout"][:, :], vout[:, :])

            def stage_cols(li):
                # ScalarE: densify the per-pixel cumsum end columns
                F = FS[li]
                o = LOFF[li]
                Q = QT[li]
                nc.scalar.activation(c15[:, o:o + F], Q[:, 15::32], ACT.Copy)
                nc.scalar.activation(c31[:, o:o + F], Q[:, 31::32], ACT.Copy)

            def trio(li, staged=True):
                F = FS[li]
                o = LOFF[li]
                rb = N_LEVEL_COLS + 3 * li
                scr = SCR[li]
                if staged:
                    i15, i31 = c15[:, o:o + F], c31[:, o:o + F]
                else:
                    Q = QT[li]
                    i15, i31 = Q[:, 15::32], Q[:, 31::32]
                nc.vector._custom_dve(
                    MULR, out=scr[:, :], in0=i15,
                    in1=w2[:, o:o + F], s0=0.0, accum_out=stats[:, rb:rb + 1])
                nc.vector._custom_dve(
                    MULR, out=scr[:, 0:F - 1], in0=i31[:, 0:F - 1],
                    in1=w2[:, o + 1:o + F], s0=0.0,
                    accum_out=stats[:, rb + 1:rb + 2])
                nc.vector._custom_dve(
                    MULR, out=scr[:, :], in0=i31,
                    in1=w2[:, o:o + F], s0=0.0,
                    accum_out=stats[:, rb + 2:rb + 3])

            # Emission order = Tile's dependency order; interleave engines
            # following the dataflow (scalar writers before DVE readers).
            qpiece(0, 0)
            # ScalarE: stage gtm to bf16 (exact) so mask TTs run at 2x;
            # sgn reads the fp8 original directly.
            nc.scalar.activation(gtmb[:, :], GTMA[:, :], ACT.Copy)
            masks()
            nc.scalar.activation(sgn[:, :], GTMA[:, 0:2 * FT],
                                 ACT.Identity, bias=1.0, scale=-2.0)
            qpiece(0, 1)
            nc.scalar.activation(clsb[:, :], CLSA[:, :], ACT.Copy)
            qpiece(0, 2)
            dd()
            qpiece(0, 3)
            # ScalarE: exp + ln; ce+1 via ln(e*x + e) = 1 + ln(1+x). The tcl
            # half (cem1) goes straight into vout with a per-level accum of
            # sum(ce_tcl + 1) (host subtracts the pixel count).
            nc.scalar.activation(expd[:, :], dce[:, :], ACT.Exp)
            nc.scalar.activation(cep1[:, :], expd[:, 0:FT], ACT.Ln,
                                 bias=econst[:, :], scale=econst[:, :])
            for li in range(3):
                F = FS[li]
                o = LOFF[li]
                nc.scalar.activation(
                    vout[:, 2 * FT + o:2 * FT + o + F],
                    expd[:, FT + o:FT + o + F],
                    ACT.Ln, bias=econst[:, :], scale=econst[:, :],
                    accum_out=stats[:, li:li + 1])
            qpiece(0, 4)
            qpiece(0, 5)
            v_ops()
            qpiece(0, 6)
            stage_cols(0)
            qpiece(1, 0)
            stage_cols(1)
            trio(0)
            qpiece(2, 0)
            trio(1)
            trio(2, staged=False)

            nc.scalar.dma_start(dram_out["stats"][:, :], stats[:, :])

    nc.compile()
    return nc


def prep_core_inputs(inputs, core):
    """Shard + relayout one core's inputs."""
    b0 = core * B_PER_CORE
    out = {}
    gtm_ch = [[], [], []]          # tr, tcl, train blocks per level
    cls_ch = [[], [], [], []]      # tr_lo, tcl_lo, tr_hi, tcl_hi
    CLS_ORDER = [0, 2, 1, 3]       # channel idx for (tr_lo, tcl_lo, tr_hi, tcl_hi)
    for li, (lvl, H, W, npc) in enumerate(LEVELS):
        F = FS[li]

        def chan(X, c):
            # one channel -> [128, F]
            return (X[:, c].reshape(B_PER_CORE * H * W)
                    .reshape(128, F))

        cls = np.asarray(inputs[f"cls{lvl}"][b0:b0 + B_PER_CORE])
        gt = np.asarray(inputs[f"gt{lvl}"][b0:b0 + B_PER_CORE])
        reg = np.asarray(inputs[f"reg{lvl}"][b0:b0 + B_PER_CORE])
        # NOTE: pixel flat order must match chan():
        # X[:, c] is [B, H, W] -> reshape(B*H*W) -> [128, F] row-major.
        for ci in range(3):
            gtm_ch[ci].append(chan(gt, ci))
        for k, ci in enumerate(CLS_ORDER):
            cls_ch[k].append(chan(cls, ci))

        # regression: per piece [128, Fp*32] channel-innermost, gtx|reg concat
        def ki_full(X):
            C = X.shape[1]          # 32
            return X.transpose(1, 0, 2, 3).reshape(C, 128, F)

        gx = ki_full(gt[:, 3:35])
        rg = ki_full(reg)
        off = 0
        for j, Fp in enumerate(PIECE_PIX[li]):
            gxp = gx[:, :, off:off + Fp].transpose(1, 2, 0).reshape(128, -1)
            rgp = rg[:, :, off:off + Fp].transpose(1, 2, 0).reshape(128, -1)
            out[f"ch{lvl}_{j}"] = np.ascontiguousarray(np.concatenate(
                [gxp, rgp], axis=-1).astype(NP_FP8))
            off += Fp

    out["gtma"] = np.ascontiguousarray(np.concatenate(
        [np.concatenate(blocks, axis=1) for blocks in gtm_ch],
        axis=1).astype(NP_FP8))
    out["clsa"] = np.ascontiguousarray(np.concatenate(
        [np.concatenate(blocks, axis=1) for blocks in cls_ch],
        axis=1).astype(NP_FP8))
    return out


def finish_host(results):
    """Merge per-core device partials into the final [4] loss vector."""
    total = np.zeros(4, dtype=np.float64)
    for li, (lvl, H, W, npc) in enumerate(LEVELS):
        F = FS[li]
        o = LOFF[li]
        n_pos_i = neg_cnt_i = 0
        loss_pos = tcl_pos = tcl_all = accx = accy = 0.0
        neg_vals = []
        for r in results:
            st = np.asarray(r["stats"], dtype=np.float64)
            tcl_all += st[:, li].sum()
            rb = N_LEVEL_COLS + 3 * li
            A = st[:, rb].sum()
            B = st[:, rb + 1].sum()
            D = st[:, rb + 2].sum()
            accx += A - B
            accy += D - A
            vo = np.asarray(r["vout"])
            w1 = vo[:, o:o + F].astype(np.float32).ravel()
            vp = vo[:, FT + o:FT + o + F].astype(np.float32).ravel()
            cm = vo[:, 2 * FT + o:2 * FT + o + F].astype(np.float32).ravel()
            pos_sel = vp > 0.0
            neg_sel = (w1 > 0.0) & ~pos_sel
            neg_vals.append(w1[neg_sel] - 1.0)
            n_pos_i += int(pos_sel.sum())
            neg_cnt_i += int(neg_sel.sum())
            loss_pos += float((vp[pos_sel] - 1.0).astype(np.float64).sum())
            tcl_pos += float((cm[pos_sel] - 1.0).astype(np.float64).sum())
        neg_vals = np.concatenate(neg_vals) if neg_vals else np.zeros(0, np.float32)
        # stats[:, li] accumulated sum(ce_tcl + 1); remove the +1 per pixel
        tcl_all -= NCORES * 128 * F

        M = 16 * H * W
        if n_pos_i > 0:
            n_neg = min(neg_cnt_i,
                        int(np.floor(np.float32(OHEM_RATIO) * np.float32(n_pos_i))))
        else:
            n_neg = 100
        k = min(n_neg, neg_vals.size)
        if k > 0:
            loss_neg = float(np.partition(neg_vals, neg_vals.size - k)
                             [neg_vals.size - k:].astype(np.float64).sum())
        else:
            loss_neg = 0.0
        loss_tr = (loss_pos + loss_neg) / (n_pos_i + float(n_neg))

        if n_pos_i > 0:
            mean_pos = tcl_pos / max(n_pos_i, 1)
            mean_neg = (tcl_all - tcl_pos) / max(M - n_pos_i, 1)
            loss_tcl = mean_pos + 0.5 * mean_neg
            denom = max(n_pos_i, 1) * KCH
            loss_rx = 0.25 * accx / denom
            loss_ry = 0.25 * accy / denom
        else:
            loss_tcl = loss_rx = loss_ry = 0.0
        total += np.array([loss_tr, loss_tcl, loss_rx, loss_ry])
    return total.astype(np.float32)


_NC_CACHE = None


def _get_nc():
    global _NC_CACHE
    if _NC_CACHE is None:
        _NC_CACHE = build_bass()
    return _NC_CACHE


def run_device(in_maps, trace=False):
    from concourse.bass_utils import run_bass_kernel_spmd
    nc = _get_nc()
    return run_bass_kernel_spmd(nc, in_maps, list(range(NCORES)), trace=trace)


def kernel(**inputs) -> np.ndarray:
    in_maps = [prep_core_inputs(inputs, c) for c in range(NCORES)]
    res = run_device(in_maps)
    return finish_host(res.results)
